# revision 15
# baseline (speedup 1.0000x reference)
"""Trainium2 Bass kernel for nn_CriticNetwork (gnn_message_passing).

Math (exact refactor of the reference):
  oa = [obs | act],  op = [obs | pol]                      # [B,N,144]
  k = oa @ Wk.T ; q = oa @ Wq.T                            # [B,N,128]
  scores[b,i,j] = k[b,i] . q[b,j] ;  w = sigmoid(scores/sqrt(128))
  value[b,i,m]  = e[b,i] + (s[b,i] + w[b,i,m] * d[b,m]) / N
     e[b,i] = u . obs[b,i]            u   = Wenc.T @ Wfin[0,:128]
     s[b,i] = sum_j w[b,i,j] va[b,j]  va  = voa . oa[b,j],  voa = Wv.T @ Wfin[0,128:]
     d[b,m] = vact . (pol-act)[b,m]   vact = voa[128:]
  weights5[b,i,m,j,0] = w[b,i,j]     (broadcast over m)

Sharding: data-parallel over batch, 16 batches per core on 8 cores.
Each core processes batches in pairs (2x64=128 partitions): even batch of a
pair lives on partitions 0:64, odd on 64:128; cross-batch blocks of the
[128,128] matmuls are computed but ignored (only diagonal blocks used).

The dominant cost is the weights5 write (1 MB/batch).  w rows are
replicated 8x in SBUF ([64,512] = 2KB rows) and DMA'd with a step-0
repeat AP so each 1 MB store uses 2 KB descriptors.
"""

import math
import sys
from contextlib import ExitStack

for _p in ("/opt/trn_rl_repo", "/opt/pypackages"):
    if _p not in sys.path:
        sys.path.insert(0, _p)

import numpy as np

import concourse.bass as bass  # noqa: F401  (dtype/AP helpers)
import concourse.tile as tile
from concourse import bacc, mybir
from concourse.bass_utils import run_bass_kernel_spmd

B, N, OBS, ACT = 128, 64, 128, 16
D, DK = OBS + ACT, 128
NCORES = 8
BPC = B // NCORES  # batches per core
PAIRS = BPC // 2
F32 = mybir.dt.float32
INV_SQRT_DK = 1.0 / math.sqrt(DK)

# test.py can flip these before calling kernel() to capture an NTFF trace.
TRACE = False
TRACE_KW = {}

_CACHE = {}


def _build_nc():
    nc = bacc.Bacc("TRN2", target_bir_lowering=False, debug=False,
                   num_devices=NCORES)
    obs_d = nc.dram_tensor("obs", [BPC, N, OBS], F32, kind="ExternalInput").ap()
    act_d = nc.dram_tensor("act", [BPC, N, ACT], F32, kind="ExternalInput").ap()
    pol_d = nc.dram_tensor("pol", [BPC, N, ACT], F32, kind="ExternalInput").ap()
    wkT_d = nc.dram_tensor("wkT", [D, DK], F32, kind="ExternalInput").ap()
    wqT_d = nc.dram_tensor("wqT", [D, DK], F32, kind="ExternalInput").ap()
    u_d = nc.dram_tensor("u", [OBS, 1], F32, kind="ExternalInput").ap()
    voa0_d = nc.dram_tensor("voa0", [OBS, 1], F32, kind="ExternalInput").ap()
    voa1_d = nc.dram_tensor("voa1", [ACT, 1], F32, kind="ExternalInput").ap()
    vrep_d = nc.dram_tensor("vrep", [ACT, 128], F32, kind="ExternalInput").ap()
    nvrep_d = nc.dram_tensor("nvrep", [ACT, 128], F32, kind="ExternalInput").ap()
    ident_d = nc.dram_tensor("ident", [64, 64], F32, kind="ExternalInput").ap()
    val_d = nc.dram_tensor("valout", [BPC, N, N], F32, kind="ExternalOutput").ap()
    w5_d = nc.dram_tensor("w5out", [BPC, N, N, N], F32, kind="ExternalOutput").ap()

    AF = mybir.ActivationFunctionType
    AX = mybir.AxisListType

    with tile.TileContext(nc) as tc, ExitStack() as ctx:
        consts = ctx.enter_context(tc.tile_pool(name="consts", bufs=1))
        sb = ctx.enter_context(tc.tile_pool(name="sb", bufs=3))
        ps_tr = ctx.enter_context(tc.tile_pool(name="ps_tr", bufs=2, space="PSUM"))
        ps_kq = ctx.enter_context(tc.tile_pool(name="ps_kq", bufs=2, space="PSUM"))
        ps_sc = ctx.enter_context(tc.tile_pool(name="ps_sc", bufs=2, space="PSUM"))
        ps_ms = ctx.enter_context(tc.tile_pool(name="ps_ms", bufs=2, space="PSUM"))

        ident = consts.tile([64, 64], F32)
        nc.sync.dma_start(ident, ident_d)
        ones1 = consts.tile([1, 128], F32)
        nc.vector.memset(ones1, 1.0)

        wk0 = consts.tile([128, DK], F32)
        nc.sync.dma_start(wk0, wkT_d[0:OBS, :])
        wk1 = consts.tile([16, DK], F32)
        nc.sync.dma_start(wk1, wkT_d[OBS:D, :])
        wq0 = consts.tile([128, DK], F32)
        nc.sync.dma_start(wq0, wqT_d[0:OBS, :])
        wq1 = consts.tile([16, DK], F32)
        nc.sync.dma_start(wq1, wqT_d[OBS:D, :])
        u_sb = consts.tile([128, 1], F32)
        nc.sync.dma_start(u_sb, u_d)
        voa0_sb = consts.tile([128, 1], F32)
        nc.sync.dma_start(voa0_sb, voa0_d)
        voa1_sb = consts.tile([16, 1], F32)
        nc.sync.dma_start(voa1_sb, voa1_d)
        vrep_sb = consts.tile([16, 128], F32)
        nc.sync.dma_start(vrep_sb, vrep_d)
        nvrep_sb = consts.tile([16, 128], F32)
        nc.sync.dma_start(nvrep_sb, nvrep_d)

        # Even/odd batch staging in separate 64-partition tiles: PE matmul
        # operands must start at partition 0 on this HW (base-partition-64
        # stationary/moving reads hard-fault the exec unit).
        obs_ev = consts.tile([64, PAIRS, OBS], F32)
        obs_od = consts.tile([64, PAIRS, OBS], F32)
        r = obs_d.rearrange("(g two) i d -> two i g d", two=2)
        nc.sync.dma_start(obs_ev, r[0])
        nc.sync.dma_start(obs_od, r[1])
        act_ev = consts.tile([64, PAIRS, ACT], F32)
        act_od = consts.tile([64, PAIRS, ACT], F32)
        r = act_d.rearrange("(g two) i c -> two i g c", two=2)
        nc.sync.dma_start(act_ev, r[0])
        nc.sync.dma_start(act_od, r[1])
        pol_ev = consts.tile([64, PAIRS, ACT], F32)
        pol_od = consts.tile([64, PAIRS, ACT], F32)
        r = pol_d.rearrange("(g two) i c -> two i g c", two=2)
        nc.sync.dma_start(pol_ev, r[0])
        nc.sync.dma_start(pol_od, r[1])

        for g in range(PAIRS):
            b0, b1 = 2 * g, 2 * g + 1

            # Transposes: obsT [d,i2], aT/pT [c,i2] (feature-major).
            # Half-transposes (64 input partitions each) so every PE
            # instruction waits on at most one DMA lane + one engine sem
            # (gen3 encodes a limited number of sync waits per matmul).
            tr = ps_tr.tile([128, 384], F32, tag="tr")
            nc.tensor.transpose(tr[:, 0:64], obs_ev[:, g, :], ident)
            nc.tensor.transpose(tr[:, 64:128], obs_od[:, g, :], ident)
            nc.tensor.transpose(tr[0:16, 128:192], act_ev[:, g, :], ident)
            nc.tensor.transpose(tr[0:16, 192:256], act_od[:, g, :], ident)
            nc.tensor.transpose(tr[0:16, 256:320], pol_ev[:, g, :], ident)
            nc.tensor.transpose(tr[0:16, 320:384], pol_od[:, g, :], ident)
            obsT = sb.tile([128, 128], F32, tag="obsT")
            nc.vector.tensor_copy(obsT, tr[:, 0:128])
            aT = sb.tile([16, 128], F32, tag="aT")
            nc.vector.tensor_copy(aT, tr[0:16, 128:256])
            pT = sb.tile([16, 128], F32, tag="pT")
            nc.vector.tensor_copy(pT, tr[0:16, 256:384])

            # kT/qT = W @ oa.T  (accumulate obs-part + act-part)
            kq = ps_kq.tile([128, 256], F32, tag="kq")
            nc.tensor.matmul(kq[:, 0:128], wk0, obsT, start=True, stop=False)
            nc.tensor.matmul(kq[:, 0:128], wk1, aT, start=False, stop=True)
            nc.tensor.matmul(kq[:, 128:256], wq0, obsT, start=True, stop=False)
            nc.tensor.matmul(kq[:, 128:256], wq1, aT, start=False, stop=True)
            kT = sb.tile([128, 128], F32, tag="kT")
            nc.vector.tensor_copy(kT, kq[:, 0:128])
            qT = sb.tile([128, 128], F32, tag="qT")
            nc.vector.tensor_copy(qT, kq[:, 128:256])

            # scores[i2,j2] = k[i].q[j]; diagonal 64x64 blocks are per-batch.
            sc = ps_sc.tile([128, 128], F32, tag="sc")
            nc.tensor.matmul(sc, kT, qT, start=True, stop=True)
            w2 = sb.tile([128, 128], F32, tag="w2")
            nc.scalar.activation(w2, sc, AF.Sigmoid, scale=INV_SQRT_DK)

            # misc bank: Vab | Db | va_row | e_col
            ms = ps_ms.tile([128, 512], F32, tag="ms")
            nc.tensor.matmul(ms[0:1, 256:384], voa0_sb, obsT, start=True, stop=False)
            nc.tensor.matmul(ms[0:1, 256:384], voa1_sb, aT, start=False, stop=True)
            va_row = sb.tile([1, 128], F32, tag="va_row")
            nc.vector.tensor_copy(va_row, ms[0:1, 256:384])
            nc.tensor.matmul(ms[:, 0:128], ones1, va_row, start=True, stop=True)
            nc.tensor.matmul(ms[:, 384:385], obsT, u_sb, start=True, stop=True)
            nc.tensor.matmul(ms[:, 128:256], vrep_sb, pT, start=True, stop=False)
            nc.tensor.matmul(ms[:, 128:256], nvrep_sb, aT, start=False, stop=True)

            # s[i] = sum_j w[i,j]*va[j] over the batch's own j-block
            tmp = sb.tile([128, 128], F32, tag="tmp")
            nc.vector.tensor_mul(tmp, w2, ms[:, 0:128])
            scol = sb.tile([128, 1], F32, tag="scol")
            nc.vector.reduce_sum(out=scol[0:64, :], in_=tmp[0:64, 0:64], axis=AX.X)
            nc.vector.reduce_sum(out=scol[64:128, :], in_=tmp[64:128, 64:128], axis=AX.X)
            colb = sb.tile([128, 1], F32, tag="colb")
            nc.vector.tensor_add(colb, scol, ms[:, 384:385])

            # value = w*d/N + colb
            prod = sb.tile([128, 128], F32, tag="prod")
            nc.vector.tensor_mul(prod, w2, ms[:, 128:256])
            val2 = sb.tile([128, 128], F32, tag="val2")
            nc.scalar.activation(val2, prod, AF.Identity, bias=colb)
            nc.gpsimd.dma_start(val_d[b0], val2[0:64, 0:64])
            nc.gpsimd.dma_start(val_d[b1], val2[64:128, 64:128])

            # weights5: replicate each w row 8x (2KB) then store 1MB/batch
            # with a step-0 x8 repeat on the source.
            w8 = sb.tile([128, 8, 64], F32, tag="w8")
            nc.vector.tensor_copy(
                w8[0:64], w2[0:64, 0:64].unsqueeze(1).broadcast_to((64, 8, 64))
            )
            nc.vector.tensor_copy(
                w8[64:128], w2[64:128, 64:128].unsqueeze(1).broadcast_to((64, 8, 64))
            )
            src0 = (w8[0:64].rearrange("p a b -> p (a b)")
                    .unsqueeze(1).broadcast_to((64, 8, 512)))
            src1 = (w8[64:128].rearrange("p a b -> p (a b)")
                    .unsqueeze(1).broadcast_to((64, 8, 512)))
            nc.sync.dma_start(w5_d[b0], src0)
            nc.sync.dma_start(w5_d[b1], src1)

    nc.compile()
    return nc


def _get_nc():
    if "nc" not in _CACHE:
        _CACHE["nc"] = _build_nc()
    return _CACHE["nc"]


def _host_consts(Wk, Wq, Wv, Wenc, Wfin):
    wkT = np.ascontiguousarray(np.asarray(Wk, np.float32).T)
    wqT = np.ascontiguousarray(np.asarray(Wq, np.float32).T)
    wf = np.asarray(Wfin, np.float64).reshape(-1)
    u = (np.asarray(Wenc, np.float64).T @ wf[:OBS]).astype(np.float32)
    voa = (np.asarray(Wv, np.float64).T @ wf[OBS:]) / float(N)
    voa0 = voa[:OBS].astype(np.float32).reshape(OBS, 1)
    voa1 = voa[OBS:].astype(np.float32).reshape(ACT, 1)
    vact = voa[OBS:].astype(np.float32).reshape(ACT, 1)
    vrep = np.ascontiguousarray(np.repeat(vact, 128, axis=1))
    return {
        "wkT": wkT, "wqT": wqT, "u": u.reshape(OBS, 1),
        "voa0": voa0, "voa1": voa1, "vrep": vrep,
        "nvrep": np.ascontiguousarray(-vrep),
        "ident": np.eye(64, dtype=np.float32),
    }


def kernel(observations, policies, actions, Wk, Wq, Wv, Wenc, Wfin):
    obs = np.ascontiguousarray(np.asarray(observations, np.float32))
    pol = np.ascontiguousarray(np.asarray(policies, np.float32))
    act = np.ascontiguousarray(np.asarray(actions, np.float32))
    cst = _host_consts(Wk, Wq, Wv, Wenc, Wfin)

    nc = _get_nc()
    in_maps = []
    for c in range(NCORES):
        sl = slice(c * BPC, (c + 1) * BPC)
        m = {"obs": obs[sl], "act": act[sl], "pol": pol[sl]}
        m.update(cst)
        in_maps.append(m)

    res = run_bass_kernel_spmd(nc, in_maps, core_ids=list(range(NCORES)),
                               trace=TRACE, **TRACE_KW)
    _CACHE["last_results"] = res
    value = np.concatenate([r["valout"] for r in res.results], axis=0)
    w5 = np.concatenate([r["w5out"] for r in res.results], axis=0)
    return (value.reshape(B, N, N, 1), w5.reshape(B, N, N, N, 1))
